# revision 22
# baseline (speedup 1.0000x reference)
"""SAGAN self-attention block on 8 TRN2 NeuronCores.

Reference (per batch element b, N = H*W = 4096, C = 512, D = 64):
    f = x @ Wf + bf ; g = x @ Wg + bg ; h = x @ Wh + bh      # [N, D]
    s = f @ g.T                                              # [N, N]
    attn = softmax(s, axis=-1)
    ctx = attn @ h                                           # [N, D]
    o = (gamma * ctx) @ Wv + bv + x                          # [N, C]

Sharding: data-parallel over batch B=8 -> one batch element per core, no
collectives. Weights replicated.

Device algorithm (per core), all matmuls bf16 with f32 PSUM accumulation:
  - load x [4096, 512] f32; cast to bf16; transpose to xT via DMA-xbar
  - f/g projections col-packed on the PE (tile_position (0,0)/(0,64)):
    fT lands on PSUM partitions 0:64, gT on 64:128; both [64, 4096] halves
    are then mirrored into the other partition half of FT2/GT2 via
    SBUF->SBUF DMA so QK pairs can row-pack.
  - h_aug[m, :] = [x@Wh + bh, 1.0]  -> [4096, 65] bf16 (m on partitions)
  - unnormalized softmax (no max subtraction: |s| <~ 50 so exp fits f32/bf16):
      for each n-chunk of 512 columns:
        for each pair (i0, i1) of 128-row m-tiles:
          S'[i0]|S'[i1] computed CONCURRENTLY via K=64 row-packing
          (tile_position (0,0) and (64,0)) into one [128, 1024] PSUM tile
          E' = exp(S')  (ScalarE, one 1024-wide call)  -> SBUF bf16
          ctxT[0:65, chunk] += haug[i0].T @ E'[:, :512] + haug[i1].T @ E'[:, 512:]
        row 64 of ctxT = sum_m E' = softmax denominator (ones column trick)
  - out[n, :] = (ctxT[:, n].T @ [gamma*Wv ; bv]) * (1/denom[n]) + x[n, :]
      (bv rides on the denom row so it survives the 1/denom scaling)
"""

import numpy as np
import ml_dtypes

BF16 = ml_dtypes.bfloat16

B, HH, WW, C = 8, 64, 64, 512
D = C // 8          # 64
N_FULL = HH * WW    # 4096
P = 128
CC = C // P         # 4  (c-chunks of 128)

_CACHE: dict = {}


def _build(n: int):
    """Build + compile the single-core Bass program (same NEFF on all 8 cores)."""
    import concourse.mybir as mybir
    from concourse import bacc
    from concourse.tile import TileContext

    f32 = mybir.dt.float32
    bf16 = mybir.dt.bfloat16
    ADD = mybir.AluOpType.add
    MULT = mybir.AluOpType.mult
    EXP = mybir.ActivationFunctionType.Exp

    n_tiles = n // P
    n_pairs = n_tiles // 2
    nch = n // 512          # number of 512-wide n-chunks

    nc = bacc.Bacc("TRN2", target_bir_lowering=False, debug=False)

    x_d = nc.dram_tensor("x", [n, C], f32, kind="ExternalInput")
    wfg_d = nc.dram_tensor("wfg", [C, 2 * D], bf16, kind="ExternalInput")
    wh_d = nc.dram_tensor("wh", [C, D], bf16, kind="ExternalInput")
    bfg_d = nc.dram_tensor("bfg", [P, 1], f32, kind="ExternalInput")   # [bf;bg]
    bh_d = nc.dram_tensor("bhp", [P, 1], f32, kind="ExternalInput")
    wv_d = nc.dram_tensor("wv", [D + 1, C], bf16, kind="ExternalInput")
    id_d = nc.dram_tensor("ident", [P, P], bf16, kind="ExternalInput")
    out_d = nc.dram_tensor("out", [n, C], f32, kind="ExternalOutput")

    x_t = x_d.rearrange("(i p) c -> i p c", p=P)
    o_t = out_d.rearrange("(i p) c -> i p c", p=P)

    with TileContext(nc) as tc:
        with (
            tc.tile_pool(name="const", bufs=1) as cpool,
            tc.tile_pool(name="big", bufs=1) as bigpool,
            tc.tile_pool(name="xb", bufs=4) as xpool,
            tc.tile_pool(name="ep", bufs=3) as epool,
            tc.tile_pool(name="ct", bufs=2) as ctpool,
            tc.tile_pool(name="os", bufs=3) as opool,
            tc.tile_pool(name="sm", bufs=4) as smpool,
            tc.tile_pool(name="psA", bufs=2, space="PSUM") as psA,
            tc.tile_pool(name="psB", bufs=2, space="PSUM") as psB,
            tc.tile_pool(name="psC", bufs=2, space="PSUM") as psC,
        ):
            # ---- replicated constants -> SBUF
            wfg_sb = cpool.tile([P, CC, 2 * D], bf16)
            nc.gpsimd.dma_start(wfg_sb, wfg_d.rearrange("(cc p) d -> p cc d", p=P))
            wh_sb = cpool.tile([P, CC, D], bf16)
            nc.gpsimd.dma_start(wh_sb, wh_d.rearrange("(cc p) d -> p cc d", p=P))
            bfg_sb = cpool.tile([P, 1], f32)
            nc.gpsimd.dma_start(bfg_sb, bfg_d[:, :])
            bh_sb = cpool.tile([P, 1], f32)
            nc.gpsimd.dma_start(bh_sb, bh_d[:, :])
            wv_sb = cpool.tile([D + 1, C], bf16)
            nc.gpsimd.dma_start(wv_sb, wv_d[:, :])
            id_sb = cpool.tile([P, P], bf16)
            nc.gpsimd.dma_start(id_sb, id_d[:, :])
            idf_sb = cpool.tile([P, P], f32)
            nc.vector.tensor_copy(out=idf_sb, in_=id_sb)

            # ---- persistent SBUF tensors
            xres = bigpool.tile([P, n_tiles, C], f32)    # x rows (residual + cast src)
            xT = bigpool.tile([P, CC, n], bf16)          # x transposed (c on partitions)
            FT2 = bigpool.tile([P, n], bf16)             # f.T duplicated in both halves
            GT2 = bigpool.tile([P, n], bf16)             # g.T duplicated in both halves
            haug = bigpool.tile([P, n_tiles, D + 1], bf16)
            nc.gpsimd.memset(haug[:, :, D:D + 1], 1.0)

            # ---- prologue: load x (f32), cast to bf16, transpose via DMA xbar.
            # All transposes ride the SyncE HWDGE queues exclusively; every
            # copy-mode DMA goes through GpSimd SWDGE queues instead, so the
            # xbar never mode-switches against copy DMAs (that serialization
            # is what sank the first DMA-transpose attempt).
            for i in range(n_tiles):
                nc.gpsimd.dma_start(xres[:, i, :], x_t[i])
                xb = xpool.tile([P, C], bf16, tag="xb")
                nc.vector.tensor_copy(out=xb, in_=xres[:, i, :])
                for cc in range(CC):
                    nc.sync.dma_start_transpose(
                        xT[:, cc, i * P:(i + 1) * P], xb[:, cc * P:(cc + 1) * P]
                    )

            # ---- f/g projections, col-packed: f -> psum rows 0:64, g -> 64:128
            for jc in range(nch):
                sl = slice(jc * 512, (jc + 1) * 512)
                fg = psA.tile([P, 512], f32, tag="sp")
                for cc in range(CC):
                    nc.tensor.matmul(
                        fg, lhsT=wfg_sb[:, cc, :], rhs=xT[:, cc, sl],
                        start=(cc == 0), stop=(cc == CC - 1),
                    )
                nc.vector.tensor_scalar(FT2[0:D, sl], fg[0:D, :], bfg_sb[0:D], None, ADD)
                nc.vector.tensor_scalar(GT2[D:P, sl], fg[D:P, :], bfg_sb[D:P], None, ADD)
                # mirror into the other partition half (SBUF->SBUF DMA)
                nc.gpsimd.dma_start(FT2[D:P, sl], FT2[0:D, sl])
                nc.gpsimd.dma_start(GT2[0:D, sl], GT2[D:P, sl])

            # ---- h projection (m on partitions) + bias via K=1 matmul
            # hT [64, n] computed with N=512 streams, col-packed two chunks
            # per PSUM tile-pair (even chunk -> partitions 0:64 via col group 0,
            # odd chunk -> partitions 64:128 via col group 64), then PE-transposed
            # back to the [m, d] layout PV needs.
            hTs = bigpool.tile([P, ((nch + 1) // 2) * 512], bf16)
            for j in range(nch):
                rh = j % 2
                jp = j // 2
                rows = slice(rh * D, rh * D + D)
                hps = psA.tile([P, 512], f32, tag="sp")
                for cc in range(CC):
                    nc.tensor.matmul(
                        hps[rows, :], lhsT=wh_sb[:, cc, :],
                        rhs=xT[:, cc, j * 512:(j + 1) * 512],
                        start=(cc == 0), stop=(cc == CC - 1),
                        tile_position=(0, rh * D),
                    )
                nc.vector.tensor_scalar(
                    hTs[rows, jp * 512:(jp + 1) * 512], hps[rows, :],
                    bh_sb[rows], None, ADD)
            for i in range(n_tiles):
                j, o = (i * P) // 512, (i * P) % 512
                rh, jp = j % 2, j // 2
                rows = slice(rh * D, rh * D + D)
                tph = psA.tile([P, D], bf16, tag="sp")
                nc.tensor.transpose(
                    tph, hTs[rows, jp * 512 + o: jp * 512 + o + P],
                    id_sb[rows, rows])
                nc.vector.tensor_copy(out=haug[:, i, 0:D], in_=tph)

            # ---- attention main loop: n-chunks of 512, m-tiles in packed pairs
            for jc in range(nch):
                sl = slice(jc * 512, (jc + 1) * 512)
                ctx = psB.tile([D + 1, 512], f32, tag="ctx")
                for ip in range(n_pairs):
                    i0, i1 = 2 * ip, 2 * ip + 1
                    sp = psA.tile([P, 1024], f32, tag="sp")
                    # two K=64 QK matmuls run concurrently in array row groups
                    nc.tensor.matmul(
                        sp[:, 0:512],
                        lhsT=GT2[0:D, i0 * P:(i0 + 1) * P], rhs=FT2[0:D, sl],
                        start=True, stop=True, tile_position=(0, 0),
                    )
                    nc.tensor.matmul(
                        sp[:, 512:1024],
                        lhsT=GT2[D:P, i1 * P:(i1 + 1) * P], rhs=FT2[D:P, sl],
                        start=True, stop=True, tile_position=(D, 0),
                    )
                    ep = epool.tile([P, 1024], bf16, tag="ep")
                    nc.scalar.activation(ep, sp, EXP)
                    nc.tensor.matmul(
                        ctx, lhsT=haug[:, i0, :], rhs=ep[:, 0:512],
                        start=(ip == 0), stop=False,
                    )
                    nc.tensor.matmul(
                        ctx, lhsT=haug[:, i1, :], rhs=ep[:, 512:1024],
                        start=False, stop=(ip == n_pairs - 1),
                    )

                # ---- epilogue for this n-chunk (4 subtiles of 128 rows)
                ct = ctpool.tile([D + 1, 512], bf16, tag="ct")
                nc.vector.tensor_copy(out=ct, in_=ctx)
                for t in range(4):
                    it = jc * 4 + t
                    tsl = slice(t * P, (t + 1) * P)
                    dt = psC.tile([P, 1], bf16, tag="oc")
                    nc.tensor.transpose(dt, ct[D:D + 1, tsl], id_sb[D:D + 1, D:D + 1])
                    rc = smpool.tile([P, 1], f32, tag="rc")
                    nc.vector.reciprocal(rc, dt)
                    op = psC.tile([P, C], f32, tag="oc")
                    nc.tensor.matmul(op, lhsT=ct[:, tsl], rhs=wv_sb, start=True, stop=True)
                    osb = opool.tile([P, C], f32, tag="os")
                    nc.vector.tensor_scalar(osb, op, rc, None, MULT)
                    nc.vector.tensor_tensor(osb, osb, xres[:, it, :], ADD)
                    nc.gpsimd.dma_start(o_t[it], osb)

    nc.compile()
    return nc


def get_program(n: int = N_FULL):
    if n not in _CACHE:
        _CACHE[n] = _build(n)
    return _CACHE[n]


def make_weight_maps(Wf, bf, Wg, bg, Wh, bh, Wv, bv, gamma):
    """Host-side layout prep of the tiny replicated weights."""
    wv_aug = np.concatenate(
        [np.float32(gamma) * np.asarray(Wv, np.float32),
         np.asarray(bv, np.float32)[None, :]], axis=0)
    bfg = np.concatenate(
        [np.asarray(bf, np.float32), np.asarray(bg, np.float32)]).reshape(P, 1)
    wfg = np.concatenate(
        [np.asarray(Wf, np.float32), np.asarray(Wg, np.float32)], axis=1)
    return {
        "wfg": np.ascontiguousarray(wfg.astype(BF16)),
        "wh": np.ascontiguousarray(np.asarray(Wh, np.float32).astype(BF16)),
        "bfg": np.ascontiguousarray(bfg),
        "bhp": np.ascontiguousarray(np.concatenate(
            [np.asarray(bh, np.float32)] * 2).reshape(P, 1)),
        "wv": np.ascontiguousarray(wv_aug.astype(BF16)),
        "ident": np.ascontiguousarray(np.eye(P, dtype=BF16)),
    }


def kernel(x, Wf, bf, Wg, bg, Wh, bh, Wv, bv, gamma):
    from concourse.bass_utils import run_bass_kernel_spmd

    x = np.asarray(x, np.float32)
    b, hh, ww, c = x.shape
    n = hh * ww
    assert (b, c) == (B, C)

    nc = get_program(n)
    base = make_weight_maps(Wf, bf, Wg, bg, Wh, bh, Wv, bv, gamma)
    xf = x.reshape(b, n, c)
    in_maps = [dict(base, x=np.ascontiguousarray(xf[i])) for i in range(b)]

    res = run_bass_kernel_spmd(nc, in_maps, core_ids=list(range(b)))
    out = np.stack([res.results[i]["out"] for i in range(b)], axis=0)
    return np.ascontiguousarray(out.reshape(b, hh, ww, c).astype(np.float32))


# revision 23
# speedup vs baseline: 1.7877x; 1.7877x over previous
"""SAGAN self-attention block on 8 TRN2 NeuronCores.

Reference (per batch element b, N = H*W = 4096, C = 512, D = 64):
    f = x @ Wf + bf ; g = x @ Wg + bg ; h = x @ Wh + bh      # [N, D]
    s = f @ g.T                                              # [N, N]
    attn = softmax(s, axis=-1)
    ctx = attn @ h                                           # [N, D]
    o = (gamma * ctx) @ Wv + bv + x                          # [N, C]

Sharding: data-parallel over batch B=8 -> one batch element per core, no
collectives. Weights replicated.

Device algorithm (per core), all matmuls bf16 with f32 PSUM accumulation:
  - load x [4096, 512] f32; cast to bf16; transpose to xT via DMA-xbar
  - f/g projections col-packed on the PE (tile_position (0,0)/(0,64)):
    fT lands on PSUM partitions 0:64, gT on 64:128; both [64, 4096] halves
    are then mirrored into the other partition half of FT2/GT2 via
    SBUF->SBUF DMA so QK pairs can row-pack.
  - h_aug[m, :] = [x@Wh + bh, 1.0]  -> [4096, 65] bf16 (m on partitions)
  - unnormalized softmax (no max subtraction: |s| <~ 50 so exp fits f32/bf16):
      for each n-chunk of 512 columns:
        for each pair (i0, i1) of 128-row m-tiles:
          S'[i0]|S'[i1] computed CONCURRENTLY via K=64 row-packing
          (tile_position (0,0) and (64,0)) into one [128, 1024] PSUM tile
          E' = exp(S')  (ScalarE, one 1024-wide call)  -> SBUF bf16
          ctxT[0:65, chunk] += haug[i0].T @ E'[:, :512] + haug[i1].T @ E'[:, 512:]
        row 64 of ctxT = sum_m E' = softmax denominator (ones column trick)
  - out[n, :] = (ctxT[:, n].T @ [gamma*Wv ; bv]) * (1/denom[n]) + x[n, :]
      (bv rides on the denom row so it survives the 1/denom scaling)
"""

import numpy as np
import ml_dtypes

BF16 = ml_dtypes.bfloat16

B, HH, WW, C = 8, 64, 64, 512
D = C // 8          # 64
N_FULL = HH * WW    # 4096
P = 128
CC = C // P         # 4  (c-chunks of 128)

_CACHE: dict = {}


def _build(n: int):
    """Build + compile the single-core Bass program (same NEFF on all 8 cores)."""
    import concourse.mybir as mybir
    from concourse import bacc
    from concourse.tile import TileContext

    f32 = mybir.dt.float32
    bf16 = mybir.dt.bfloat16
    ADD = mybir.AluOpType.add
    MULT = mybir.AluOpType.mult
    EXP = mybir.ActivationFunctionType.Exp

    n_tiles = n // P
    n_pairs = n_tiles // 2
    nch = n // 512          # number of 512-wide n-chunks

    nc = bacc.Bacc("TRN2", target_bir_lowering=False, debug=False)

    x_d = nc.dram_tensor("x", [n, C], f32, kind="ExternalInput")
    wfg_d = nc.dram_tensor("wfg", [C, 2 * D], bf16, kind="ExternalInput")
    wh_d = nc.dram_tensor("wh", [C, D], bf16, kind="ExternalInput")
    bfg_d = nc.dram_tensor("bfg", [P, 1], f32, kind="ExternalInput")   # [bf;bg]
    bh_d = nc.dram_tensor("bhp", [P, 1], f32, kind="ExternalInput")
    wv_d = nc.dram_tensor("wv", [D + 1, C], bf16, kind="ExternalInput")
    id_d = nc.dram_tensor("ident", [P, P], bf16, kind="ExternalInput")
    out_d = nc.dram_tensor("out", [n, C], f32, kind="ExternalOutput")

    x_t = x_d.rearrange("(i p) c -> i p c", p=P)
    o_t = out_d.rearrange("(i p) c -> i p c", p=P)

    with TileContext(nc) as tc:
        with (
            tc.tile_pool(name="const", bufs=1) as cpool,
            tc.tile_pool(name="big", bufs=1) as bigpool,
            tc.tile_pool(name="ep", bufs=3) as epool,
            tc.tile_pool(name="ct", bufs=2) as ctpool,
            tc.tile_pool(name="os", bufs=3) as opool,
            tc.tile_pool(name="sm", bufs=4) as smpool,
            tc.tile_pool(name="psA", bufs=2, space="PSUM") as psA,
            tc.tile_pool(name="psB", bufs=2, space="PSUM") as psB,
            tc.tile_pool(name="psC", bufs=2, space="PSUM") as psC,
        ):
            # ---- replicated constants -> SBUF
            wfg_sb = cpool.tile([P, CC, 2 * D], bf16)
            nc.sync.dma_start(wfg_sb, wfg_d.rearrange("(cc p) d -> p cc d", p=P))
            wh_sb = cpool.tile([P, CC, D], bf16)
            nc.sync.dma_start(wh_sb, wh_d.rearrange("(cc p) d -> p cc d", p=P))
            bfg_sb = cpool.tile([P, 1], f32)
            nc.sync.dma_start(bfg_sb, bfg_d[:, :])
            bh_sb = cpool.tile([P, 1], f32)
            nc.sync.dma_start(bh_sb, bh_d[:, :])
            wv_sb = cpool.tile([D + 1, C], bf16)
            nc.sync.dma_start(wv_sb, wv_d[:, :])
            id_sb = cpool.tile([P, P], bf16)
            nc.sync.dma_start(id_sb, id_d[:, :])
            idf_sb = cpool.tile([P, P], f32)
            nc.vector.tensor_copy(out=idf_sb, in_=id_sb)

            # ---- persistent SBUF tensors
            xres = bigpool.tile([P, n_tiles, C], f32)    # x rows (residual + cast src)
            xT = bigpool.tile([P, CC, n], bf16)          # x transposed (c on partitions)
            FT2 = bigpool.tile([P, n], bf16)             # f.T duplicated in both halves
            GT2 = bigpool.tile([P, n], bf16)             # g.T duplicated in both halves
            haug = bigpool.tile([P, n_tiles, D + 1], bf16)
            nc.gpsimd.memset(haug[:, :, D:D + 1], 1.0)

            # ---- prologue: load x in 128x128 quarters, transpose f32 on the
            # PE (each transpose fires as soon as its quarter lands), cast to
            # bf16 on the PSUM->SBUF copy.
            for i in range(n_tiles):
                tp = psA.tile([P, C], f32, tag="sp")
                for cc in range(CC):
                    csl = slice(cc * P, (cc + 1) * P)
                    nc.sync.dma_start(xres[:, i, csl], x_t[i][:, csl])
                    nc.tensor.transpose(
                        tp[:, cc * P:(cc + 1) * P], xres[:, i, csl], idf_sb
                    )
                nc.vector.tensor_copy(
                    out=xT[:, :, i * P:(i + 1) * P],
                    in_=tp.rearrange("p (cc q) -> p cc q", q=P),
                )

            # ---- f/g projections, col-packed: f -> psum rows 0:64, g -> 64:128
            for jc in range(nch):
                sl = slice(jc * 512, (jc + 1) * 512)
                fg = psA.tile([P, 512], f32, tag="sp")
                for cc in range(CC):
                    nc.tensor.matmul(
                        fg, lhsT=wfg_sb[:, cc, :], rhs=xT[:, cc, sl],
                        start=(cc == 0), stop=(cc == CC - 1),
                    )
                nc.vector.tensor_scalar(FT2[0:D, sl], fg[0:D, :], bfg_sb[0:D], None, ADD)
                nc.vector.tensor_scalar(GT2[D:P, sl], fg[D:P, :], bfg_sb[D:P], None, ADD)
                # mirror into the other partition half (SBUF->SBUF DMA)
                nc.sync.dma_start(FT2[D:P, sl], FT2[0:D, sl])
                nc.sync.dma_start(GT2[0:D, sl], GT2[D:P, sl])

            # ---- h projection (m on partitions) + bias via K=1 matmul
            # hT [64, n] computed with N=512 streams, col-packed two chunks
            # per PSUM tile-pair (even chunk -> partitions 0:64 via col group 0,
            # odd chunk -> partitions 64:128 via col group 64), then PE-transposed
            # back to the [m, d] layout PV needs.
            hTs = bigpool.tile([P, ((nch + 1) // 2) * 512], bf16)
            for j in range(nch):
                rh = j % 2
                jp = j // 2
                rows = slice(rh * D, rh * D + D)
                hps = psA.tile([P, 512], f32, tag="sp")
                for cc in range(CC):
                    nc.tensor.matmul(
                        hps[rows, :], lhsT=wh_sb[:, cc, :],
                        rhs=xT[:, cc, j * 512:(j + 1) * 512],
                        start=(cc == 0), stop=(cc == CC - 1),
                        tile_position=(0, rh * D),
                    )
                nc.vector.tensor_scalar(
                    hTs[rows, jp * 512:(jp + 1) * 512], hps[rows, :],
                    bh_sb[rows], None, ADD)
            for i in range(n_tiles):
                j, o = (i * P) // 512, (i * P) % 512
                rh, jp = j % 2, j // 2
                rows = slice(rh * D, rh * D + D)
                tph = psA.tile([P, D], bf16, tag="sp")
                nc.tensor.transpose(
                    tph, hTs[rows, jp * 512 + o: jp * 512 + o + P],
                    id_sb[rows, rows])
                nc.vector.tensor_copy(out=haug[:, i, 0:D], in_=tph)

            # ---- attention main loop: n-chunks of 512, m-tiles in packed pairs
            for jc in range(nch):
                sl = slice(jc * 512, (jc + 1) * 512)
                ctx = psB.tile([D + 1, 512], f32, tag="ctx")
                for ip in range(n_pairs):
                    i0, i1 = 2 * ip, 2 * ip + 1
                    sp = psA.tile([P, 1024], f32, tag="sp")
                    # two K=64 QK matmuls run concurrently in array row groups
                    nc.tensor.matmul(
                        sp[:, 0:512],
                        lhsT=GT2[0:D, i0 * P:(i0 + 1) * P], rhs=FT2[0:D, sl],
                        start=True, stop=True, tile_position=(0, 0),
                    )
                    nc.tensor.matmul(
                        sp[:, 512:1024],
                        lhsT=GT2[D:P, i1 * P:(i1 + 1) * P], rhs=FT2[D:P, sl],
                        start=True, stop=True, tile_position=(D, 0),
                    )
                    ep = epool.tile([P, 1024], bf16, tag="ep")
                    nc.scalar.activation(ep, sp, EXP)
                    nc.tensor.matmul(
                        ctx, lhsT=haug[:, i0, :], rhs=ep[:, 0:512],
                        start=(ip == 0), stop=False,
                    )
                    nc.tensor.matmul(
                        ctx, lhsT=haug[:, i1, :], rhs=ep[:, 512:1024],
                        start=False, stop=(ip == n_pairs - 1),
                    )

                # ---- epilogue for this n-chunk (4 subtiles of 128 rows)
                ct = ctpool.tile([D + 1, 512], bf16, tag="ct")
                nc.vector.tensor_copy(out=ct, in_=ctx)
                for t in range(4):
                    it = jc * 4 + t
                    tsl = slice(t * P, (t + 1) * P)
                    dt = psC.tile([P, 1], bf16, tag="oc")
                    nc.tensor.transpose(dt, ct[D:D + 1, tsl], id_sb[D:D + 1, D:D + 1])
                    rc = smpool.tile([P, 1], f32, tag="rc")
                    nc.vector.reciprocal(rc, dt)
                    op = psC.tile([P, C], f32, tag="oc")
                    nc.tensor.matmul(op, lhsT=ct[:, tsl], rhs=wv_sb, start=True, stop=True)
                    osb = opool.tile([P, C], f32, tag="os")
                    nc.vector.tensor_scalar(osb, op, rc, None, MULT)
                    nc.vector.tensor_tensor(osb, osb, xres[:, it, :], ADD)
                    nc.sync.dma_start(o_t[it], osb)

    nc.compile()
    return nc


def get_program(n: int = N_FULL):
    if n not in _CACHE:
        _CACHE[n] = _build(n)
    return _CACHE[n]


def make_weight_maps(Wf, bf, Wg, bg, Wh, bh, Wv, bv, gamma):
    """Host-side layout prep of the tiny replicated weights."""
    wv_aug = np.concatenate(
        [np.float32(gamma) * np.asarray(Wv, np.float32),
         np.asarray(bv, np.float32)[None, :]], axis=0)
    bfg = np.concatenate(
        [np.asarray(bf, np.float32), np.asarray(bg, np.float32)]).reshape(P, 1)
    wfg = np.concatenate(
        [np.asarray(Wf, np.float32), np.asarray(Wg, np.float32)], axis=1)
    return {
        "wfg": np.ascontiguousarray(wfg.astype(BF16)),
        "wh": np.ascontiguousarray(np.asarray(Wh, np.float32).astype(BF16)),
        "bfg": np.ascontiguousarray(bfg),
        "bhp": np.ascontiguousarray(np.concatenate(
            [np.asarray(bh, np.float32)] * 2).reshape(P, 1)),
        "wv": np.ascontiguousarray(wv_aug.astype(BF16)),
        "ident": np.ascontiguousarray(np.eye(P, dtype=BF16)),
    }


def kernel(x, Wf, bf, Wg, bg, Wh, bh, Wv, bv, gamma):
    from concourse.bass_utils import run_bass_kernel_spmd

    x = np.asarray(x, np.float32)
    b, hh, ww, c = x.shape
    n = hh * ww
    assert (b, c) == (B, C)

    nc = get_program(n)
    base = make_weight_maps(Wf, bf, Wg, bg, Wh, bh, Wv, bv, gamma)
    xf = x.reshape(b, n, c)
    in_maps = [dict(base, x=np.ascontiguousarray(xf[i])) for i in range(b)]

    res = run_bass_kernel_spmd(nc, in_maps, core_ids=list(range(b)))
    out = np.stack([res.results[i]["out"] for i in range(b)], axis=0)
    return np.ascontiguousarray(out.reshape(b, hh, ww, c).astype(np.float32))
